# revision 10
# baseline (speedup 1.0000x reference)
"""Trainium2 Bass kernel for nn_DenoisingNet_1580547972055.

The reference computes out[batch, i] = ELU(W[0, i] + b[0]) broadcast over the
batch dimension -- the values of input_list are never read, only its shape
matters.  So the kernel computes a 1024-element ELU once per core and writes a
broadcast (batch_shard, 1024) f32 block to HBM.  Sharding: batch axis split
8 ways (8192 rows per core); W/b replicated; no collectives needed.

ELU is composed from available ACT functions without catastrophic cancellation:
    m   = min(x, 0) = -relu(-x)
    elu = relu(x) + tanh(m/2) * (exp(m) + 1)      # tanh(m/2)*(e^m+1) == e^m-1

Instruction graph is shaped so no instruction needs more than one embedded
sync wait (this walrus target rejects 2+ waits per instruction): each DMA
completion is absorbed by a dedicated same-engine op before fan-out, and all
ACT biases are explicit SBUF APs (a float bias would pull in a const-AP
preamble dependency).
"""

import os

import numpy as np

L = 1024
B = 65536
N_CORES = 8
B_SHARD = B // N_CORES  # 8192
P = 128

# Output-write strategy, overridable for A/B profiling:
#   bigtile: replicate vals NREP times per partition in SBUF, then
#            (B_SHARD//(P*NREP)) DMAs each moving P*NREP rows.
#   bcast:   step-0 (broadcast) source AP; NDMA DMAs re-reading the same
#            [128, 1024] SBUF tile.
#   plain:   B_SHARD//P DMAs of [128, 1024] (512 KB each).
VARIANT = os.environ.get("KERNEL_VARIANT", "bigtile")
NREP = int(os.environ.get("KERNEL_NREP", "8"))
NDMA = int(os.environ.get("KERNEL_NDMA", "8"))
DUAL_RING = os.environ.get("KERNEL_DUAL_RING", "0") == "1"
# small: compute ELU on a [128, 8] layout (free-dim 8 -> ~50ns ACT ops instead
# of ~1.1us at free-dim 1024), then round-trip through DRAM to broadcast the
# 1024-vector to all 128 partitions.
SMALL_COMPUTE = os.environ.get("KERNEL_SMALL", "1") == "1"

_cache = {}


def _legalize_multiwaits(nc):
    """This walrus build allows at most ONE embedded sync-wait per
    instruction; Tile emits several (same-engine pipeline RAW + DMA sems,
    and the tail drain aggregates everything).  Split extras into standalone
    single-wait NoOps placed immediately before the instruction on the same
    engine -- semantically identical (per-engine program order)."""
    import concourse.mybir as mybir

    for fn in nc.m.functions:
        for bl in fn.blocks:
            new_insts = []
            for inst in bl.instructions:
                si = inst.sync_info
                if si is not None and si.on_wait and len(si.on_wait) > 1:
                    waits = list(si.on_wait)
                    for w in waits[:-1]:
                        new_insts.append(
                            mybir.InstNoOp(
                                name=nc.get_next_instruction_name(),
                                ins=[],
                                outs=[],
                                engine=inst.engine,
                                sync_info=mybir.SyncInfo(on_wait=[w], on_update=[]),
                                bass_nofuse=True,
                            )
                        )
                    si.on_wait = [waits[-1]]
                new_insts.append(inst)
            bl.instructions = new_insts


def _build_raw():
    """Raw-bass version: no TileContext preamble barriers / tail butterfly.
    Explicit semaphores; every wait is a standalone single-sem instruction."""
    from concourse import bass, mybir

    f32 = mybir.dt.float32
    Act = mybir.ActivationFunctionType

    nc = bass.Bass(enable_partition_id=False)
    W = nc.declare_dram_parameter("W", [1, L], f32, isOutput=False)
    b = nc.declare_dram_parameter("b", [1, 1], f32, isOutput=False)
    out = nc.declare_dram_parameter("out", [B_SHARD, L], f32, isOutput=True)
    scratch = nc.dram_tensor("scratch", [1, L], f32)

    CW = L // P  # 8 elements per partition for the small compute

    with (
        nc.sbuf_tensor([P, CW], f32) as wt,
        nc.sbuf_tensor([P, 1], f32) as bt,
        nc.sbuf_tensor([P, 1], f32) as zt,
        nc.sbuf_tensor([P, CW], f32) as xt,
        nc.sbuf_tensor([P, CW], f32) as r,
        nc.sbuf_tensor([P, CW], f32) as mneg,
        nc.sbuf_tensor([P, CW], f32) as t,
        nc.sbuf_tensor([P, CW], f32) as e,
        nc.sbuf_tensor([P, CW], f32) as s,
        nc.sbuf_tensor([P, CW], f32) as q,
        nc.sbuf_tensor([P, CW], f32) as vsmall,
        nc.sbuf_tensor([P, L], f32) as vals,
        nc.semaphore("s_in") as s_in,
        nc.semaphore("s_dve") as s_dve,
        nc.semaphore("s_act") as s_act,
        nc.semaphore("s_sc") as s_sc,
        nc.semaphore("s_vl") as s_vl,
        nc.semaphore("s_out") as s_out,
        nc.Block() as block,
    ):

        @block.sync
        def _(sync):
            sync.dma_start(
                out=wt[:], in_=W.rearrange("o (p j) -> (o p) j", p=P)
            ).then_inc(s_in, 16)
            sync.dma_start(out=bt[:], in_=b[0:1, :].to_broadcast((P, 1))).then_inc(
                s_in, 16
            )
            sync.wait_ge(s_dve, 5)  # vsmall ready
            sync.dma_start(
                out=scratch.rearrange("o (p j) -> (o p) j", p=P), in_=vsmall[:]
            ).then_inc(s_sc, 16)
            sync.wait_ge(s_sc, 16)
            sync.dma_start(
                out=vals[:], in_=scratch[0:1, :].to_broadcast((P, L))
            ).then_inc(s_vl, 16)
            sync.wait_ge(s_vl, 16)
            rows = B_SHARD // NDMA
            j = rows // P
            for i in range(NDMA):
                ov = out[i * rows : (i + 1) * rows, :].rearrange(
                    "(p j) m -> p j m", p=P
                )
                src = vals[:].unsqueeze(1).to_broadcast((P, j, L))
                sync.dma_start(out=ov, in_=src).then_inc(s_out, 16)
            sync.wait_ge(s_out, 16 * NDMA)

        @block.vector
        def _(vector):
            nc.vector.memset(zt[:], 0.0).then_inc(s_dve, 1)  # -> 1
            vector.wait_ge(s_in, 32)
            nc.vector.tensor_scalar_add(xt[:], wt[:], bt[:]).then_inc(s_dve, 1)  # 2
            vector.wait_ge(s_act, 4)  # r, mneg, t, e all done
            nc.vector.tensor_scalar_add(s[:], e[:], 1.0).then_inc(s_dve, 1)  # 3
            vector.wait_ge(s_dve, 3)  # s landed (same-engine RAW)
            nc.vector.tensor_mul(q[:], t[:], s[:]).then_inc(s_dve, 1)  # 4
            vector.wait_ge(s_dve, 4)  # q landed
            nc.vector.tensor_add(vsmall[:], r[:], q[:]).then_inc(s_dve, 1)  # 5

        @block.scalar
        def _(scalar):
            scalar.wait_ge(s_dve, 2)  # zt + xt
            nc.scalar.activation(r[:], xt[:], Act.Relu, bias=zt[:], scale=1.0).then_inc(
                s_act, 1
            )
            nc.scalar.activation(
                mneg[:], xt[:], Act.Relu, bias=zt[:], scale=-1.0
            ).then_inc(s_act, 1)
            scalar.wait_ge(s_act, 2)  # mneg landed (same-engine RAW)
            nc.scalar.activation(
                t[:], mneg[:], Act.Tanh, bias=zt[:], scale=-0.5
            ).then_inc(s_act, 1)
            nc.scalar.activation(
                e[:], mneg[:], Act.Exp, bias=zt[:], scale=-1.0
            ).then_inc(s_act, 1)

    _legalize_multiwaits(nc)
    return nc


def _build_bass():
    from concourse import bass, mybir, tile

    f32 = mybir.dt.float32
    Act = mybir.ActivationFunctionType

    nc = bass.Bass(enable_partition_id=False)
    W = nc.declare_dram_parameter("W", [1, L], f32, isOutput=False)
    b = nc.declare_dram_parameter("b", [1, 1], f32, isOutput=False)
    out = nc.declare_dram_parameter("out", [B_SHARD, L], f32, isOutput=True)
    scratch = nc.dram_tensor("scratch", [1, L], f32) if SMALL_COMPUTE else None

    with tile.TileContext(nc) as tc:
        with tc.tile_pool(name="pool", bufs=1) as pool:
            CW = L // P if SMALL_COMPUTE else L  # compute-tile free dim
            wt = pool.tile([P, CW], f32)
            if SMALL_COMPUTE:
                # W as [128, 8]: partition p holds W[8p:8p+8]
                nc.sync.dma_start(
                    out=wt[:], in_=W.rearrange("o (p j) -> (o p) j", p=P)
                )
            else:
                nc.sync.dma_start(out=wt[:], in_=W[0:1, :].to_broadcast((P, L)))
            bt = pool.tile([P, 1], f32)
            nc.sync.dma_start(out=bt[:], in_=b[0:1, :].to_broadcast((P, 1)))

            zt = pool.tile([P, 1], f32)  # explicit zero bias for ACT ops
            nc.vector.memset(zt[:], 0.0)
            btc = pool.tile([P, 1], f32)  # absorbs the b-DMA wait on DVE
            nc.vector.tensor_copy(btc[:], bt[:])
            xt = pool.tile([P, CW], f32)  # x = W + b  (waits only on W-DMA)
            nc.vector.tensor_scalar_add(xt[:], wt[:], btc[:])

            r = pool.tile([P, CW], f32)  # relu(x)
            nc.scalar.activation(r[:], xt[:], Act.Relu, bias=zt[:], scale=1.0)
            mneg = pool.tile([P, CW], f32)  # relu(-x) = -min(x, 0)
            nc.scalar.activation(mneg[:], xt[:], Act.Relu, bias=zt[:], scale=-1.0)
            t = pool.tile([P, CW], f32)  # tanh(min(x,0)/2)
            nc.scalar.activation(t[:], mneg[:], Act.Tanh, bias=zt[:], scale=-0.5)
            e = pool.tile([P, CW], f32)  # exp(min(x,0))
            nc.scalar.activation(e[:], mneg[:], Act.Exp, bias=zt[:], scale=-1.0)

            s = pool.tile([P, CW], f32)
            nc.vector.tensor_scalar_add(s[:], e[:], 1.0)
            q = pool.tile([P, CW], f32)
            nc.vector.tensor_mul(q[:], t[:], s[:])
            vsmall = pool.tile([P, CW], f32)
            nc.vector.tensor_add(vsmall[:], r[:], q[:])

            if SMALL_COMPUTE:
                # Round-trip through DRAM to broadcast the 1024-vector from
                # partition-major [128, 8] layout to every partition.
                nc.sync.dma_start(
                    out=scratch.rearrange("o (p j) -> (o p) j", p=P), in_=vsmall[:]
                )
                vals = pool.tile([P, L], f32)
                nc.sync.dma_start(
                    out=vals[:], in_=scratch[0:1, :].to_broadcast((P, L))
                )
            else:
                vals = vsmall

            if VARIANT == "bigtile":
                big = pool.tile([P, NREP * L], f32)
                for j in range(NREP):
                    nc.vector.tensor_copy(big[:, j * L : (j + 1) * L], vals[:])
                rows = P * NREP
                n_dma = B_SHARD // rows
                for i in range(n_dma):
                    ov = out[i * rows : (i + 1) * rows, :].rearrange(
                        "(p j) m -> p (j m)", p=P
                    )
                    eng = nc.scalar if (DUAL_RING and i % 2 == 1) else nc.sync
                    eng.dma_start(out=ov, in_=big[:])
            elif VARIANT == "bcast":
                rows = B_SHARD // NDMA  # rows per DMA
                j = rows // P  # broadcast repeat per partition
                for i in range(NDMA):
                    ov = out[i * rows : (i + 1) * rows, :].rearrange(
                        "(p j) m -> p j m", p=P
                    )
                    src = vals[:].unsqueeze(1).to_broadcast((P, j, L))
                    eng = nc.scalar if (DUAL_RING and i % 2 == 1) else nc.sync
                    eng.dma_start(out=ov, in_=src)
            elif VARIANT == "plain":
                for i in range(B_SHARD // P):
                    eng = nc.scalar if (DUAL_RING and i % 2 == 1) else nc.sync
                    eng.dma_start(out=out[i * P : (i + 1) * P, :], in_=vals[:])
            else:
                raise ValueError(f"unknown variant {VARIANT}")

    _legalize_multiwaits(nc)
    return nc


def _get_nc():
    key = (VARIANT, NREP, NDMA, DUAL_RING, SMALL_COMPUTE)
    if key not in _cache:
        _cache[key] = _build_raw() if VARIANT == "raw" else _build_bass()
    return _cache[key]


def run_sharded(W, b, trace=False, trace_cores=None):
    """Run the SPMD kernel; returns (full_output, BassKernelResults)."""
    from concourse.bass_utils import run_bass_kernel_spmd

    nc = _get_nc()
    Wf = np.ascontiguousarray(np.asarray(W, dtype=np.float32).reshape(1, L))
    bf = np.ascontiguousarray(np.asarray(b, dtype=np.float32).reshape(1, 1))
    in_maps = [{"W": Wf, "b": bf} for _ in range(N_CORES)]
    res = run_bass_kernel_spmd(
        nc,
        in_maps,
        core_ids=list(range(N_CORES)),
        trace=trace,
        trace_cores=trace_cores,
    )
    full = np.concatenate([r["out"] for r in res.results], axis=0)
    return full, res


def kernel(input_list, W, b):
    assert input_list.shape == (L, B)
    full, _ = run_sharded(W, b, trace=False)
    return full
